# revision 26
# baseline (speedup 1.0000x reference)
"""Trainium2 Bass kernel for the nn_Discriminator feasibility-probability model.

Strategy (pure data parallel over 8 cores, 8192 rows each):
  - One [B,500] @ [500,548] matmul per core carries almost everything:
      cols   0:500  -> dQ = d @ Omega   (bias row folds the -x_bw@Omega shift)
      cols 500:546  -> 46 "threshold" columns a_k = w_k@x + b_k such that
                       relu(a_k) are exactly the relu(...) constraint terms for
                       sum-to-one, sector, mq and beta-neutrality constraints
      col  546      -> l2 = d @ alpha
    An appended ones-column of x provides the bias row.
  - Per 128-row tile: PE transposes x (matmul needs features on partitions),
    fp32r matmuls (full rate at N>=256), then fused vector/scalar ops:
      sumabs via tensor_scalar(abs_max, accum), dQd via tensor_tensor_reduce,
      nnz via ACT Tanh(scale=1000, accum), group-relu sum via ACT Relu(accum).
  - Final batched pass combines per-row stats into pre-tanh `tot`.
  - Host applies the global l_scalar term and the final tanh with XLA's
    fp32 saturation semantics (tanh(t)=1 for t>7.90531), then unshards.
"""

import numpy as np

import concourse.bass as bass
import concourse.tile as tile
from concourse import mybir
from concourse.bass_utils import run_bass_kernel_spmd

B, D = 65536, 500
NCORES = 8
R = B // NCORES            # rows per core
P = 128                    # partitions / rows per tile
T = R // P                 # tiles per core (64)
DA = D + 1                 # augmented feature count (ones column)
NW = 548                   # W columns: 500 Omega + 46 thresholds + l2 + pad
BANK = 274                 # psum bank split (both halves >=256 for fp32r rate)
NG = 46                    # threshold (relu) columns
# feature chunking for the 128-wide PE transpose: 501 = 126 + 125*3
CHUNKS = [(0, 126), (126, 251), (251, 376), (376, 501)]

F32 = mybir.dt.float32
F32R = mybir.dt.float32r
AF = mybir.ActivationFunctionType
OP = mybir.AluOpType

_CACHED = {}


def _build_weight_matrix(x_bw, alpha, beta, Omega, sector_mask, mq_mask):
    """[DA, NW] fp32: folded weights + bias row (row 500)."""
    W = np.zeros((DA, NW), dtype=np.float32)
    W[0:D, 0:D] = Omega
    W[D, 0:D] = -(x_bw @ Omega)

    cols_w = []
    cols_b = []

    def hi_lo(w, c, thr):
        # value = x@w - c; emit relu(value - thr) and relu(-value - thr)
        cols_w.append(w)
        cols_b.append(-c - thr)
        cols_w.append(-w)
        cols_b.append(c - thr)

    ones = np.ones(D, dtype=np.float64)
    # relu(s-1) + relu(1-s), s = sum(x):  value = x@ones, c = 0, thr = +-1
    cols_w.append(ones)
    cols_b.append(-1.0)
    cols_w.append(-ones)
    cols_b.append(1.0)
    for g in range(sector_mask.shape[0]):
        w = sector_mask[g].astype(np.float64)
        hi_lo(w, float(x_bw.astype(np.float64) @ w), 0.1)
    for g in range(mq_mask.shape[0]):
        w = mq_mask[g].astype(np.float64)
        hi_lo(w, float(x_bw.astype(np.float64) @ w), 0.1)
    bw = beta.astype(np.float64)
    hi_lo(bw, float(x_bw.astype(np.float64) @ bw), 0.1)
    assert len(cols_w) == NG
    W[0:D, D : D + NG] = np.stack(cols_w, axis=1).astype(np.float32)
    W[D, D : D + NG] = np.asarray(cols_b, dtype=np.float32)
    # l2 column
    W[0:D, D + NG] = alpha
    W[D, D + NG] = -float(x_bw.astype(np.float64) @ alpha.astype(np.float64))
    return W


def _build_program(rows=R, split_waits=True):
    T = rows // P
    nc = bass.Bass()
    xs = nc.declare_dram_parameter("xs", [rows, D], F32, isOutput=False)
    wmat = nc.declare_dram_parameter("wmat", [DA, NW], F32, isOutput=False)
    xbw = nc.declare_dram_parameter("xbw", [1, D], F32, isOutput=False)
    ident_in = nc.declare_dram_parameter("ident", [P, P], F32, isOutput=False)
    tot_out = nc.declare_dram_parameter("tot_out", [P, T], F32, isOutput=True)
    sumabs_out = nc.declare_dram_parameter("sumabs_out", [P, T], F32, isOutput=True)

    from contextlib import ExitStack
    with tile.TileContext(nc) as tc, ExitStack() as ctx:
        singles = ctx.enter_context(tc.tile_pool(name="singles", bufs=1))
        xpool = ctx.enter_context(tc.tile_pool(name="xpool", bufs=3))
        tpool = ctx.enter_context(tc.tile_pool(name="tpool", bufs=3))
        scr = ctx.enter_context(tc.tile_pool(name="scr", bufs=2))
        stats = ctx.enter_context(tc.tile_pool(name="stats", bufs=1))
        pt_pool = ctx.enter_context(tc.tile_pool(name="pt", bufs=2, space="PSUM"))
        pa_pool = ctx.enter_context(tc.tile_pool(name="pa", bufs=2, space="PSUM"))
        pb_pool = ctx.enter_context(tc.tile_pool(name="pb", bufs=2, space="PSUM"))

        # --- constants ---
        ident = singles.tile([P, P], F32)
        nc.sync.dma_start(out=ident, in_=ident_in.ap())
        xbw_bc = singles.tile([P, D], F32)
        xbw_ap = xbw.ap()
        nc.sync.dma_start(
            out=xbw_bc,
            in_=bass.AP(tensor=xbw_ap.tensor, offset=xbw_ap.offset,
                        ap=[[0, P], [1, D]]),
        )
        w_sb = []
        for (c0, c1) in CHUNKS:
            wt_raw = singles.tile([c1 - c0, NW], F32, tag=f"wraw{c0}")
            nc.sync.dma_start(out=wt_raw, in_=wmat.ap()[c0:c1, :])
            wt = singles.tile([c1 - c0, NW], F32R, tag=f"w{c0}")
            nc.vector.tensor_copy(out=wt, in_=wt_raw)
            w_sb.append(wt)

        # warm-ups: consume preamble-loaded tiles once per consuming engine so
        # steady-state instructions carry a single sync wait (the ISA compute
        # encodings have one wait slot)
        warm_ps = pt_pool.tile([P, P], F32, tag="warm")
        nc.tensor.transpose(warm_ps, ident, ident)
        warm_v = singles.tile([P, 1], F32, tag="warmv")
        nc.vector.tensor_copy(out=warm_v, in_=xbw_bc[:, 0:1])

        # --- per-row stats, one column per tile ---
        st_sumabs = stats.tile([P, T], F32)
        st_nnz = stats.tile([P, T], F32)
        st_g = stats.tile([P, T], F32)
        st_qa = stats.tile([P, T], F32)
        st_qb = stats.tile([P, T], F32)
        st_l2 = stats.tile([P, T], F32)

        for t in range(T):
            xt = xpool.tile([P, DA], F32, tag="xt")
            nc.sync.dma_start(out=xt[:, 0:D], in_=xs.ap()[t * P : (t + 1) * P, :])
            nc.vector.memset(xt[:, D : D + 1], 1.0)

            # transpose x_aug into [feature, row] chunks (psum), then to SBUF
            pt = pt_pool.tile([126, 512], F32, tag="pt")
            for c, (c0, c1) in enumerate(CHUNKS):
                nc.tensor.transpose(pt[0 : c1 - c0, c * P : (c + 1) * P],
                                    xt[:, c0:c1], ident)
            xT = tpool.tile([126, 512], F32R, tag="xT")
            nc.vector.tensor_copy(out=xT[0:126, 0:P], in_=pt[0:126, 0:P])
            nc.vector.tensor_copy(out=xT[0:125, P:512], in_=pt[0:125, P:512])

            # matmuls: psumA = x_aug @ W[:, 0:274], psumB = x_aug @ W[:, 274:548]
            pa = pa_pool.tile([P, BANK], F32, tag="pa")
            pb = pb_pool.tile([P, BANK], F32, tag="pb")
            for c, (c0, c1) in enumerate(CHUNKS):
                k = c1 - c0
                lhsT = xT[0:k, c * P : (c + 1) * P]
                nc.tensor.matmul(pa, lhsT, w_sb[c][:, 0:BANK],
                                 start=(c == 0), stop=(c == 3))
                nc.tensor.matmul(pb, lhsT, w_sb[c][:, BANK:NW],
                                 start=(c == 0), stop=(c == 3))

            # d = x - x_bw
            dt_ = xpool.tile([P, D], F32, tag="dt")
            nc.vector.tensor_tensor(out=dt_, in0=xt[:, 0:D], in1=xbw_bc,
                                    op=OP.subtract)

            # sumabs = sum |d|
            nc.vector.tensor_reduce(out=st_sumabs[:, t : t + 1], in_=dt_,
                                    axis=mybir.AxisListType.X, op=OP.add,
                                    apply_absolute_value=True)
            # dQd = sum(dQ * d) split over the two psum banks
            # (scalar_tensor_tensor is the native TensorScalarPtr encoding;
            # tensor_tensor_reduce is an extended op that wedges this runtime)
            sA = scr.tile([P, BANK], F32, tag="sA")
            nc.vector.scalar_tensor_tensor(out=sA, in0=pa, scalar=1.0,
                                           in1=dt_[:, 0:BANK], op0=OP.mult,
                                           op1=OP.mult,
                                           accum_out=st_qa[:, t : t + 1])
            sB = scr.tile([P, D - BANK], F32, tag="sB")
            nc.vector.scalar_tensor_tensor(out=sB, in0=pb[:, 0 : D - BANK],
                                           scalar=1.0, in1=dt_[:, BANK:D],
                                           op0=OP.mult, op1=OP.mult,
                                           accum_out=st_qb[:, t : t + 1])
            # nnz = sum tanh(1000 x)
            s500b = scr.tile([P, D], F32, tag="s500b")
            nc.scalar.activation(out=s500b, in_=xt[:, 0:D], func=AF.Tanh,
                                 scale=1000.0, accum_out=st_nnz[:, t : t + 1])
            # G = sum relu(threshold cols) — DVE so every PSUM reader is DVE
            # (keeps PE matmul WAR waits vector-clock-elidable)
            g46 = scr.tile([P, NG], F32, tag="g46")
            nc.vector.tensor_scalar(out=g46, in0=pb[:, D - BANK : D - BANK + NG],
                                    scalar1=0.0, scalar2=None, op0=OP.max,
                                    op1=OP.add, accum_out=st_g[:, t : t + 1])
            # l2 passthrough
            nc.vector.tensor_copy(out=st_l2[:, t : t + 1],
                                  in_=pb[:, D - BANK + NG : D - BANK + NG + 1])

        # --- final combine over [P, T] stats ---
        fin = stats.tile([P, T], F32, tag="fin")      # tot accumulator
        tmp1 = stats.tile([P, T], F32, tag="tmp1")
        tmp2 = stats.tile([P, T], F32, tag="tmp2")
        dqd = stats.tile([P, T], F32, tag="dqd")

        # lead with the ACT-produced nnz read so later DVE ops only ever wait
        # on DVE: fin = relu(nnz - 70)
        nc.vector.tensor_scalar(out=fin, in0=st_nnz, scalar1=70.0,
                                scalar2=0.0, op0=OP.subtract, op1=OP.max)
        nc.vector.tensor_tensor(out=dqd, in0=st_qa, in1=st_qb, op=OP.add)
        # += G + relu(sumabs - 0.05)
        nc.vector.tensor_scalar(out=tmp1, in0=st_sumabs, scalar1=0.05,
                                scalar2=0.0, op0=OP.subtract, op1=OP.max)
        nc.vector.tensor_tensor(out=fin, in0=fin, in1=st_g, op=OP.add)
        nc.vector.tensor_tensor(out=fin, in0=fin, in1=tmp1, op=OP.add)
        # += relu(50 - nnz) = 50 - min(nnz, 50)
        nc.vector.tensor_scalar(out=tmp1, in0=st_nnz, scalar1=50.0,
                                scalar2=None, op0=OP.min)
        nc.vector.tensor_scalar(out=tmp2, in0=tmp1, scalar1=-1.0,
                                scalar2=50.0, op0=OP.mult, op1=OP.add)
        nc.vector.tensor_tensor(out=fin, in0=fin, in1=tmp2, op=OP.add)
        # += 0.5*relu(dqd - 0.005) + 0.5*relu(0.0025 - dqd)
        nc.vector.tensor_scalar(out=tmp1, in0=dqd, scalar1=0.005,
                                scalar2=0.0, op0=OP.subtract, op1=OP.max)
        nc.vector.scalar_tensor_tensor(out=fin, in0=tmp1, scalar=0.5, in1=fin,
                                       op0=OP.mult, op1=OP.add)
        nc.vector.tensor_scalar(out=tmp1, in0=dqd, scalar1=0.0025,
                                scalar2=None, op0=OP.min)
        nc.vector.tensor_scalar(out=tmp2, in0=tmp1, scalar1=-1.0,
                                scalar2=0.0025, op0=OP.mult, op1=OP.add)
        nc.vector.scalar_tensor_tensor(out=fin, in0=tmp2, scalar=0.5, in1=fin,
                                       op0=OP.mult, op1=OP.add)
        # += 10*relu(100*(dqd - l2) - 1000)
        nc.vector.tensor_tensor(out=tmp1, in0=dqd, in1=st_l2, op=OP.subtract)
        nc.vector.tensor_scalar(out=tmp2, in0=tmp1, scalar1=100.0,
                                scalar2=1000.0, op0=OP.mult, op1=OP.subtract)
        nc.vector.tensor_scalar(out=tmp1, in0=tmp2, scalar1=0.0,
                                scalar2=None, op0=OP.max)
        nc.vector.scalar_tensor_tensor(out=fin, in0=tmp1, scalar=10.0, in1=fin,
                                       op0=OP.mult, op1=OP.add)

        nc.scalar.dma_start(out=tot_out.ap(), in_=fin)
        nc.scalar.dma_start(out=sumabs_out.ap(), in_=st_sumabs)
    # populate .instr bytes for InstISA subclasses (tensor_tensor_reduce);
    # raw Bass skips this pass and the NEFF compiler rejects empty .instr
    from concourse.library_overlay import lower_extended_insts
    lower_extended_insts(nc)
    if split_waits:
        _split_multi_waits(nc)
    return nc


def _split_multi_waits(nc):
    """This walrus build allows a single sync-wait on most instruction
    encodings; hoist extra waits onto dedicated EventSemaphore instructions
    (which queue on the same engine sequencer, preserving order)."""
    import bass_rust
    n = 0
    for fn in nc.m.functions:
        for b in fn.blocks:
            il = b.instructions
            k = 0
            while k < len(il):
                i = il[k]
                si = i.sync_info
                if si is not None and len(si.on_wait) > 1:
                    waits = list(si.on_wait)
                    for w in waits[:-1]:
                        e = mybir.InstEventSemaphore(
                            name=f"{i.name}-wsplit{n}", ins=[], outs=[])
                        n += 1
                        e.engine = i.engine
                        e.sync_info = bass_rust.SyncInfo(on_wait=[w],
                                                        on_update=[])
                        il.insert(k, e)
                        k += 1
                    i.sync_info = bass_rust.SyncInfo(
                        on_wait=[waits[-1]], on_update=list(si.on_update))
                k += 1


def _get_program():
    if "nc" not in _CACHED:
        _CACHED["nc"] = _build_program()
    return _CACHED["nc"]


def kernel(x, x_bw, alpha, beta, w_pre, Omega, sector_mask, mq_mask):
    x = np.ascontiguousarray(x, dtype=np.float32)
    W = _build_weight_matrix(
        np.asarray(x_bw, np.float32), np.asarray(alpha, np.float32),
        np.asarray(beta, np.float32), np.asarray(Omega, np.float32),
        np.asarray(sector_mask, np.float32), np.asarray(mq_mask, np.float32))
    xbw_row = np.ascontiguousarray(np.asarray(x_bw, np.float32)[None, :])

    nc = _get_program()
    ident = np.eye(P, dtype=np.float32)
    in_maps = [
        {"xs": x[c * R : (c + 1) * R], "wmat": W, "xbw": xbw_row, "ident": ident}
        for c in range(NCORES)
    ]
    res = run_bass_kernel_spmd(nc, in_maps, list(range(NCORES)))
    _CACHED["last_res"] = res

    tot = np.empty(B, dtype=np.float32)
    sumabs = np.empty(B, dtype=np.float32)
    for c in range(NCORES):
        tot[c * R : (c + 1) * R] = res.results[c]["tot_out"].T.reshape(R)
        sumabs[c * R : (c + 1) * R] = res.results[c]["sumabs_out"].T.reshape(R)

    _CACHED["last_tot"] = tot.copy()
    _CACHED["last_sumabs"] = sumabs.copy()
    # global scalar active-share term, then the final tanh with XLA fp32
    # semantics (tanh saturates to exactly 1.0 above 7.90531)
    l_scalar = np.float32(0.5) * np.float32(sumabs.sum(dtype=np.float64))
    tot = tot + np.maximum(np.float32(0.6) - l_scalar, np.float32(0))
    targ = (tot / np.float32(100.0)).astype(np.float32)
    th = np.tanh(targ, dtype=np.float32)
    th = np.where(targ > np.float32(7.90531), np.float32(1.0), th)
    out = np.maximum(np.float32(1.0) - th, np.float32(0.0))
    return out.astype(np.float32)


# revision 29
# speedup vs baseline: 1.3218x; 1.3218x over previous
"""Trainium2 Bass kernel for the nn_Discriminator feasibility-probability model.

Strategy (pure data parallel over 8 cores, 8192 rows each):
  - One [B,500] @ [500,548] matmul per core carries almost everything:
      cols   0:500  -> dQ = d @ Omega   (bias row folds the -x_bw@Omega shift)
      cols 500:546  -> 46 "threshold" columns a_k = w_k@x + b_k such that
                       relu(a_k) are exactly the relu(...) constraint terms for
                       sum-to-one, sector, mq and beta-neutrality constraints
      col  546      -> l2 = d @ alpha
    An appended ones-column of x provides the bias row.
  - Per 128-row tile: PE transposes x (matmul needs features on partitions),
    fp32r matmuls (full rate at N>=256), then fused vector/scalar ops:
      sumabs via tensor_scalar(abs_max, accum), dQd via tensor_tensor_reduce,
      nnz via ACT Tanh(scale=1000, accum), group-relu sum via ACT Relu(accum).
  - Final batched pass combines per-row stats into pre-tanh `tot`.
  - Host applies the global l_scalar term and the final tanh with XLA's
    fp32 saturation semantics (tanh(t)=1 for t>7.90531), then unshards.
"""

import numpy as np

import concourse.bass as bass
import concourse.tile as tile
from concourse import mybir
from concourse.bass_utils import run_bass_kernel_spmd

B, D = 65536, 500
NCORES = 8
R = B // NCORES            # rows per core
P = 128                    # partitions / rows per tile
T = R // P                 # tiles per core (64)
DA = D + 1                 # augmented feature count (ones column)
NW = 548                   # W columns: 500 Omega + 46 thresholds + l2 + pad
BANK = 274                 # psum bank split (both halves >=256 for fp32r rate)
NG = 46                    # threshold (relu) columns
# feature chunking for the 128-wide PE transpose: 501 = 126 + 125*3
CHUNKS = [(0, 126), (126, 251), (251, 376), (376, 501)]

F32 = mybir.dt.float32
F32R = mybir.dt.float32r
AF = mybir.ActivationFunctionType
OP = mybir.AluOpType

_CACHED = {}


def _build_weight_matrix(x_bw, alpha, beta, Omega, sector_mask, mq_mask):
    """[DA, NW] fp32: folded weights + bias row (row 500)."""
    W = np.zeros((DA, NW), dtype=np.float32)
    W[0:D, 0:D] = Omega
    W[D, 0:D] = -(x_bw @ Omega)

    cols_w = []
    cols_b = []

    def hi_lo(w, c, thr):
        # value = x@w - c; emit relu(value - thr) and relu(-value - thr)
        cols_w.append(w)
        cols_b.append(-c - thr)
        cols_w.append(-w)
        cols_b.append(c - thr)

    ones = np.ones(D, dtype=np.float64)
    # relu(s-1) + relu(1-s), s = sum(x):  value = x@ones, c = 0, thr = +-1
    cols_w.append(ones)
    cols_b.append(-1.0)
    cols_w.append(-ones)
    cols_b.append(1.0)
    for g in range(sector_mask.shape[0]):
        w = sector_mask[g].astype(np.float64)
        hi_lo(w, float(x_bw.astype(np.float64) @ w), 0.1)
    for g in range(mq_mask.shape[0]):
        w = mq_mask[g].astype(np.float64)
        hi_lo(w, float(x_bw.astype(np.float64) @ w), 0.1)
    bw = beta.astype(np.float64)
    hi_lo(bw, float(x_bw.astype(np.float64) @ bw), 0.1)
    assert len(cols_w) == NG
    W[0:D, D : D + NG] = np.stack(cols_w, axis=1).astype(np.float32)
    W[D, D : D + NG] = np.asarray(cols_b, dtype=np.float32)
    # l2 column
    W[0:D, D + NG] = alpha
    W[D, D + NG] = -float(x_bw.astype(np.float64) @ alpha.astype(np.float64))
    return W


def _build_program(rows=R, split_waits=True):
    T = rows // P
    nc = bass.Bass()
    xs = nc.declare_dram_parameter("xs", [rows, D], F32, isOutput=False)
    wmat = nc.declare_dram_parameter("wmat", [DA, NW], F32, isOutput=False)
    xbw = nc.declare_dram_parameter("xbw", [1, D], F32, isOutput=False)
    ident_in = nc.declare_dram_parameter("ident", [P, P], F32, isOutput=False)
    tot_out = nc.declare_dram_parameter("tot_out", [P, T], F32, isOutput=True)
    sumabs_out = nc.declare_dram_parameter("sumabs_out", [P, T], F32, isOutput=True)

    from contextlib import ExitStack
    with tile.TileContext(nc) as tc, ExitStack() as ctx:
        singles = ctx.enter_context(tc.tile_pool(name="singles", bufs=1))
        xpool = ctx.enter_context(tc.tile_pool(name="xpool", bufs=3))
        tpool = ctx.enter_context(tc.tile_pool(name="tpool", bufs=3))
        scr = ctx.enter_context(tc.tile_pool(name="scr", bufs=2))
        stats = ctx.enter_context(tc.tile_pool(name="stats", bufs=1))
        pt_pool = ctx.enter_context(tc.tile_pool(name="pt", bufs=2, space="PSUM"))
        pa_pool = ctx.enter_context(tc.tile_pool(name="pa", bufs=2, space="PSUM"))
        pb_pool = ctx.enter_context(tc.tile_pool(name="pb", bufs=2, space="PSUM"))

        # --- constants ---
        ident = singles.tile([P, P], F32)
        nc.sync.dma_start(out=ident, in_=ident_in.ap())
        xbw_bc = singles.tile([P, D], F32)
        xbw_ap = xbw.ap()
        nc.sync.dma_start(
            out=xbw_bc,
            in_=bass.AP(tensor=xbw_ap.tensor, offset=xbw_ap.offset,
                        ap=[[0, P], [1, D]]),
        )
        w_sb = []
        for (c0, c1) in CHUNKS:
            wt_raw = singles.tile([c1 - c0, NW], F32, tag=f"wraw{c0}")
            nc.sync.dma_start(out=wt_raw, in_=wmat.ap()[c0:c1, :])
            wt = singles.tile([c1 - c0, NW], F32R, tag=f"w{c0}")
            nc.vector.tensor_copy(out=wt, in_=wt_raw)
            w_sb.append(wt)

        # warm-ups: consume preamble-loaded tiles once per consuming engine so
        # steady-state instructions carry a single sync wait (the ISA compute
        # encodings have one wait slot)
        warm_ps = pt_pool.tile([P, P], F32, tag="warm")
        nc.tensor.transpose(warm_ps, ident, ident)
        warm_v = singles.tile([P, 1], F32, tag="warmv")
        nc.vector.tensor_copy(out=warm_v, in_=xbw_bc[:, 0:1])
        warm_g = singles.tile([P, 1], F32, tag="warmg")
        nc.gpsimd.tensor_copy(out=warm_g, in_=xbw_bc[:, 0:1])

        # --- per-row stats, one column per tile ---
        st_sumabs = stats.tile([P, T], F32)
        st_nnz = stats.tile([P, T], F32)
        st_g = stats.tile([P, T], F32)
        st_qa = stats.tile([P, T], F32)
        st_qb = stats.tile([P, T], F32)
        st_l2 = stats.tile([P, T], F32)

        for t in range(T):
            xt = xpool.tile([P, DA], F32, tag="xt")
            nc.sync.dma_start(out=xt[:, 0:D], in_=xs.ap()[t * P : (t + 1) * P, :])
            nc.gpsimd.memset(xt[:, D : D + 1], 1.0)

            # transpose x_aug into [feature, row] chunks (psum), then to SBUF
            pt = pt_pool.tile([126, 512], F32, tag="pt")
            for c, (c0, c1) in enumerate(CHUNKS):
                nc.tensor.transpose(pt[0 : c1 - c0, c * P : (c + 1) * P],
                                    xt[:, c0:c1], ident)
            xT = tpool.tile([126, 512], F32R, tag="xT")
            nc.vector.tensor_copy(out=xT[0:126, 0:P], in_=pt[0:126, 0:P])
            nc.vector.tensor_copy(out=xT[0:125, P:512], in_=pt[0:125, P:512])

            # matmuls: psumA = x_aug @ W[:, 0:274], psumB = x_aug @ W[:, 274:548]
            pa = pa_pool.tile([P, BANK], F32, tag="pa")
            pb = pb_pool.tile([P, BANK], F32, tag="pb")
            for c, (c0, c1) in enumerate(CHUNKS):
                k = c1 - c0
                lhsT = xT[0:k, c * P : (c + 1) * P]
                nc.tensor.matmul(pa, lhsT, w_sb[c][:, 0:BANK],
                                 start=(c == 0), stop=(c == 3))
                nc.tensor.matmul(pb, lhsT, w_sb[c][:, BANK:NW],
                                 start=(c == 0), stop=(c == 3))

            # d = x - x_bw  (gpsimd — keeps DVE free)
            dt_ = xpool.tile([P, D], F32, tag="dt")
            nc.gpsimd.tensor_tensor(out=dt_, in0=xt[:, 0:D], in1=xbw_bc,
                                    op=OP.subtract)

            # sumabs = sum |d|  (ACT abs + accumulate)
            sab = scr.tile([P, D], F32, tag="sab")
            nc.scalar.activation(out=sab, in_=dt_, func=AF.Abs,
                                 accum_out=st_sumabs[:, t : t + 1])
            # dQd = sum(dQ * d) split over the two psum banks
            # (scalar_tensor_tensor is the native TensorScalarPtr encoding;
            # tensor_tensor_reduce is an extended op that wedges this runtime)
            sA = scr.tile([P, BANK], F32, tag="sA")
            nc.vector.scalar_tensor_tensor(out=sA, in0=pa, scalar=1.0,
                                           in1=dt_[:, 0:BANK], op0=OP.mult,
                                           op1=OP.mult,
                                           accum_out=st_qa[:, t : t + 1])
            sB = scr.tile([P, D - BANK], F32, tag="sB")
            nc.vector.scalar_tensor_tensor(out=sB, in0=pb[:, 0 : D - BANK],
                                           scalar=1.0, in1=dt_[:, BANK:D],
                                           op0=OP.mult, op1=OP.mult,
                                           accum_out=st_qb[:, t : t + 1])
            # nnz = sum tanh(1000 x)
            s500b = scr.tile([P, D], F32, tag="s500b")
            nc.scalar.activation(out=s500b, in_=xt[:, 0:D], func=AF.Tanh,
                                 scale=1000.0, accum_out=st_nnz[:, t : t + 1])
            # G = sum relu(threshold cols) — DVE so every PSUM reader is DVE
            # (keeps PE matmul WAR waits vector-clock-elidable)
            g46 = scr.tile([P, NG], F32, tag="g46")
            nc.vector.tensor_scalar(out=g46, in0=pb[:, D - BANK : D - BANK + NG],
                                    scalar1=0.0, scalar2=None, op0=OP.max,
                                    op1=OP.add, accum_out=st_g[:, t : t + 1])
            # l2 passthrough
            nc.vector.tensor_copy(out=st_l2[:, t : t + 1],
                                  in_=pb[:, D - BANK + NG : D - BANK + NG + 1])

        # --- final combine over [P, T] stats ---
        fin = stats.tile([P, T], F32, tag="fin")      # tot accumulator
        tmp1 = stats.tile([P, T], F32, tag="tmp1")
        tmp2 = stats.tile([P, T], F32, tag="tmp2")
        dqd = stats.tile([P, T], F32, tag="dqd")

        # lead with the ACT-produced nnz read so later DVE ops only ever wait
        # on DVE: fin = relu(nnz - 70)
        nc.vector.tensor_scalar(out=fin, in0=st_nnz, scalar1=70.0,
                                scalar2=0.0, op0=OP.subtract, op1=OP.max)
        nc.vector.tensor_tensor(out=dqd, in0=st_qa, in1=st_qb, op=OP.add)
        # += G + relu(sumabs - 0.05)
        nc.vector.tensor_scalar(out=tmp1, in0=st_sumabs, scalar1=0.05,
                                scalar2=0.0, op0=OP.subtract, op1=OP.max)
        nc.vector.tensor_tensor(out=fin, in0=fin, in1=st_g, op=OP.add)
        nc.vector.tensor_tensor(out=fin, in0=fin, in1=tmp1, op=OP.add)
        # += relu(50 - nnz) = 50 - min(nnz, 50)
        nc.vector.tensor_scalar(out=tmp1, in0=st_nnz, scalar1=50.0,
                                scalar2=None, op0=OP.min)
        nc.vector.tensor_scalar(out=tmp2, in0=tmp1, scalar1=-1.0,
                                scalar2=50.0, op0=OP.mult, op1=OP.add)
        nc.vector.tensor_tensor(out=fin, in0=fin, in1=tmp2, op=OP.add)
        # += 0.5*relu(dqd - 0.005) + 0.5*relu(0.0025 - dqd)
        nc.vector.tensor_scalar(out=tmp1, in0=dqd, scalar1=0.005,
                                scalar2=0.0, op0=OP.subtract, op1=OP.max)
        nc.vector.scalar_tensor_tensor(out=fin, in0=tmp1, scalar=0.5, in1=fin,
                                       op0=OP.mult, op1=OP.add)
        nc.vector.tensor_scalar(out=tmp1, in0=dqd, scalar1=0.0025,
                                scalar2=None, op0=OP.min)
        nc.vector.tensor_scalar(out=tmp2, in0=tmp1, scalar1=-1.0,
                                scalar2=0.0025, op0=OP.mult, op1=OP.add)
        nc.vector.scalar_tensor_tensor(out=fin, in0=tmp2, scalar=0.5, in1=fin,
                                       op0=OP.mult, op1=OP.add)
        # += 10*relu(100*(dqd - l2) - 1000)
        nc.vector.tensor_tensor(out=tmp1, in0=dqd, in1=st_l2, op=OP.subtract)
        nc.vector.tensor_scalar(out=tmp2, in0=tmp1, scalar1=100.0,
                                scalar2=1000.0, op0=OP.mult, op1=OP.subtract)
        nc.vector.tensor_scalar(out=tmp1, in0=tmp2, scalar1=0.0,
                                scalar2=None, op0=OP.max)
        nc.vector.scalar_tensor_tensor(out=fin, in0=tmp1, scalar=10.0, in1=fin,
                                       op0=OP.mult, op1=OP.add)

        nc.scalar.dma_start(out=tot_out.ap(), in_=fin)
        nc.scalar.dma_start(out=sumabs_out.ap(), in_=st_sumabs)
    # populate .instr bytes for InstISA subclasses (tensor_tensor_reduce);
    # raw Bass skips this pass and the NEFF compiler rejects empty .instr
    from concourse.library_overlay import lower_extended_insts
    lower_extended_insts(nc)
    if split_waits:
        _split_multi_waits(nc)
    return nc


def _split_multi_waits(nc):
    """This walrus build allows a single sync-wait on most instruction
    encodings; hoist extra waits onto dedicated EventSemaphore instructions
    (which queue on the same engine sequencer, preserving order)."""
    import bass_rust
    n = 0
    for fn in nc.m.functions:
        for b in fn.blocks:
            il = b.instructions
            k = 0
            while k < len(il):
                i = il[k]
                si = i.sync_info
                if si is not None and len(si.on_wait) > 1:
                    waits = list(si.on_wait)
                    for w in waits[:-1]:
                        e = mybir.InstEventSemaphore(
                            name=f"{i.name}-wsplit{n}", ins=[], outs=[])
                        n += 1
                        e.engine = i.engine
                        e.sync_info = bass_rust.SyncInfo(on_wait=[w],
                                                        on_update=[])
                        il.insert(k, e)
                        k += 1
                    i.sync_info = bass_rust.SyncInfo(
                        on_wait=[waits[-1]], on_update=list(si.on_update))
                k += 1


def _get_program():
    if "nc" not in _CACHED:
        _CACHED["nc"] = _build_program()
    return _CACHED["nc"]


def kernel(x, x_bw, alpha, beta, w_pre, Omega, sector_mask, mq_mask):
    x = np.ascontiguousarray(x, dtype=np.float32)
    W = _build_weight_matrix(
        np.asarray(x_bw, np.float32), np.asarray(alpha, np.float32),
        np.asarray(beta, np.float32), np.asarray(Omega, np.float32),
        np.asarray(sector_mask, np.float32), np.asarray(mq_mask, np.float32))
    xbw_row = np.ascontiguousarray(np.asarray(x_bw, np.float32)[None, :])

    nc = _get_program()
    ident = np.eye(P, dtype=np.float32)
    in_maps = [
        {"xs": x[c * R : (c + 1) * R], "wmat": W, "xbw": xbw_row, "ident": ident}
        for c in range(NCORES)
    ]
    res = run_bass_kernel_spmd(nc, in_maps, list(range(NCORES)))
    _CACHED["last_res"] = res

    tot = np.empty(B, dtype=np.float32)
    sumabs = np.empty(B, dtype=np.float32)
    for c in range(NCORES):
        tot[c * R : (c + 1) * R] = res.results[c]["tot_out"].T.reshape(R)
        sumabs[c * R : (c + 1) * R] = res.results[c]["sumabs_out"].T.reshape(R)

    _CACHED["last_tot"] = tot.copy()
    _CACHED["last_sumabs"] = sumabs.copy()
    # global scalar active-share term, then the final tanh with XLA fp32
    # semantics (tanh saturates to exactly 1.0 above 7.90531)
    l_scalar = np.float32(0.5) * np.float32(sumabs.sum(dtype=np.float64))
    tot = tot + np.maximum(np.float32(0.6) - l_scalar, np.float32(0))
    targ = (tot / np.float32(100.0)).astype(np.float32)
    th = np.tanh(targ, dtype=np.float32)
    th = np.where(targ > np.float32(7.90531), np.float32(1.0), th)
    out = np.maximum(np.float32(1.0) - th, np.float32(0.0))
    return out.astype(np.float32)
